# revision 12
# baseline (speedup 1.0000x reference)
"""Shifted abs-diff cost volume kernel for Trainium2 (8 NeuronCores).

out[n, d, y, x] = |image1[n,0,y,x] - image2[n,0,y,x-d]|  (0 where x < d)

Sharding: pure data parallel over flattened (N*H) rows -> 96 rows/core.

Per-core layout: each row is split into 4 column quarters of 312; the
96x4 = 384 quarter-segments are packed onto all 128 SBUF partitions (3
segments per partition, partition p = 32*quarter + (row%32), slot
s = row//32). Each img2 segment carries a 127-column left halo so the
disparity shift stays inside the partition's free axis. Using all 128
partitions is what makes the output DMA run at ~360 GB/s (a
96-partition source caps at ~240 GB/s) and cuts DVE/ACT time by 4/3.

Per group of GROUP disparities: one grouped TensorTensor (raw AP,
stride -1 over the disparity axis) subtracts, GPSIMD zeroes the x<d
wedges (only the 32 partitions holding quarter 0 have any), ACT does an
in-place abs, and one DMA per group streams out on alternating HWDGE
rings.
"""

import numpy as np

import concourse.bass as bass
import concourse.tile as tile
from concourse import mybir
from concourse.ap import AP
from concourse.bass_utils import run_bass_kernel_spmd

N, C, H, W = 2, 1, 384, 1248
D = 128  # MAXDISP
NCORES = 8
ROWS = (N * H) // NCORES  # 96 rows per core
GROUP = 4  # disparities per grouped instruction / output DMA
NGROUPS = D // GROUP
Q = 4  # column quarters per row
SEG = W // Q  # 312 columns per segment
SLOTS = ROWS * Q // 128  # 3 segments per partition
HALO = D - 1  # 127
IMG1_COLS = SLOTS * SEG  # 936
HALO_COLS = SLOTS * (SEG + HALO)  # 3*439 = 1317
IN_COLS = IMG1_COLS + HALO_COLS  # 2253
TFREE = SLOTS * GROUP * SEG  # 3744 free elements per work tile
F32 = mybir.dt.float32

_NC_CACHE = {}


def build_program():
    nc = bass.Bass("TRN2", target_bir_lowering=False, debug=False)
    imgs_d = nc.dram_tensor("images", [128, IN_COLS], F32, kind="ExternalInput").ap()
    # Per-core output layout [slot, partition, d, x-in-segment]; the
    # host reassembles rows/quarters. Group DMAs write [s, p, d0:d0+G, :]
    # as 4992-byte contiguous chunks per (s, p).
    out_d = nc.dram_tensor("out", [SLOTS, 128, D * SEG], F32, kind="ExternalOutput").ap()

    with tile.TileContext(nc) as tc:
        with (
            tc.tile_pool(name="inp", bufs=1) as inp_pool,
            tc.tile_pool(name="work", bufs=10) as work_pool,
        ):
            imgs = inp_pool.tile([128, IN_COLS], F32)
            nc.sync.dma_start(out=imgs[:, :], in_=imgs_d[:, :])

            # Small leading groups shorten the serial ramp (input load ->
            # first TT -> first abs) before the first output DMA.
            sched = []
            d0 = 0
            for gsize in (1, 1, 2):
                sched.append((d0, gsize))
                d0 += gsize
            while d0 < D:
                sched.append((d0, GROUP))
                d0 += GROUP

            for g, (d0, gsize) in enumerate(sched):
                # work tile free layout (s, i, x): s stride gsize*SEG,
                # i stride SEG, x stride 1
                tfree = SLOTS * gsize * SEG
                t = work_pool.tile([128, tfree], F32, tag="t")

                # out[p, s, i, x] = img1seg[p, s, x] - img2halo[p, s, HALO + x - d0 - i]
                out_ap = AP(
                    t.tensor,
                    0,
                    [[tfree, 128], [gsize * SEG, SLOTS], [SEG, gsize], [1, SEG]],
                )
                in0 = AP(
                    imgs.tensor,
                    0,
                    [[IN_COLS, 128], [SEG, SLOTS], [0, gsize], [1, SEG]],
                )
                in1 = AP(
                    imgs.tensor,
                    IMG1_COLS + HALO - d0,
                    [[IN_COLS, 128], [SEG + HALO, SLOTS], [-1, gsize], [1, SEG]],
                )
                nc.vector.tensor_sub(out_ap, in0, in1)

                # x<d wedges exist only on quarter-0 partitions [0:32).
                # Zero them between TT and abs (|0| stays 0) so ACT's abs
                # is the single last writer and the DMA gates on ACT.
                for i in range(gsize):
                    d = d0 + i
                    if d > 0:
                        nc.gpsimd.memset(
                            AP(
                                t.tensor,
                                i * SEG,
                                [[tfree, 32], [gsize * SEG, SLOTS], [1, d]],
                            ),
                            0.0,
                        )

                # In-place abs over the whole tile.
                nc.scalar.activation(
                    t[:, :], t[:, :], mybir.ActivationFunctionType.Abs
                )

                # One DMA per group: DRAM dims ordered (p, s, cols) to
                # match the SBUF tile's (partition, slot, cols) walk.
                dram_ap = AP(
                    out_d.tensor,
                    d0 * SEG,
                    [[D * SEG, 128], [128 * D * SEG, SLOTS], [1, gsize * SEG]],
                )
                sbuf_ap = AP(
                    t.tensor,
                    0,
                    [[tfree, 128], [gsize * SEG, SLOTS], [1, gsize * SEG]],
                )
                dma_eng = nc.sync if g % 2 == 0 else nc.scalar
                dma_eng.dma_start(out=dram_ap, in_=sbuf_ap)
    return nc


def split_excess_waits(nc):
    """Split multi-wait instructions for this walrus build's ISA encoder.

    The TRN2 ISA encoding here holds 1 semaphore wait per engine
    instruction (2 for a standalone EventSemaphore). Tile's scheduler
    fuses up to ~3 waits per instruction, which this neuronxcc rejects
    with "Too many sync wait commands". Moving the excess waits into
    EventSemaphore instructions issued just before, on the same engine
    queue, is semantically identical (the engine stalls at the sync
    instruction instead).
    """
    counter = 0
    for f in nc.m.functions:
        for b in f.blocks:
            plan = []  # (index, [event_insts]) in original order
            insts = b.instructions
            for idx, inst in enumerate(insts):
                si = inst.sync_info
                if si is None:
                    continue
                waits = list(si.on_wait)
                cap = 2 if inst.opcode == "EventSemaphore" else 1
                if len(waits) <= cap:
                    continue
                extra, keep = waits[:-cap], waits[-cap:]
                evs = []
                for j in range(0, len(extra), 2):
                    ev = mybir.InstEventSemaphore(
                        name=f"EVWS-{counter}",
                        opcode="EventSemaphore",
                        engine=inst.engine,
                    )
                    counter += 1
                    ev.sync_info = mybir.SyncInfo(
                        on_wait=extra[j : j + 2], on_update=[]
                    )
                    evs.append(ev)
                inst.sync_info = mybir.SyncInfo(
                    on_wait=keep, on_update=list(si.on_update)
                )
                plan.append((idx, evs))
            # apply inserts back-to-front so earlier indices stay valid
            for idx, evs in reversed(plan):
                for k, ev in enumerate(evs):
                    insts.insert(idx + k, ev)
    return nc


def get_program():
    if "nc" not in _NC_CACHE:
        _NC_CACHE["nc"] = split_excess_waits(build_program())
    return _NC_CACHE["nc"]


def shard_inputs(image1, image2):
    img1 = np.asarray(image1, dtype=np.float32).reshape(N * H, W)
    img2 = np.asarray(image2, dtype=np.float32).reshape(N * H, W)
    # left halo pad for quarter 0
    img2p = np.concatenate(
        [np.zeros((N * H, HALO), np.float32), img2], axis=1
    )  # [rows, HALO+W]
    maps = []
    for k in range(NCORES):
        i1 = img1[k * ROWS : (k + 1) * ROWS]  # [96, 1248]
        i2 = img2p[k * ROWS : (k + 1) * ROWS]  # [96, 127+1248]
        img1seg = np.empty((128, SLOTS, SEG), np.float32)
        img2halo = np.empty((128, SLOTS, SEG + HALO), np.float32)
        p = np.arange(128)
        c, rm = p // 32, p % 32
        for s in range(SLOTS):
            r = 32 * s + rm  # [128]
            img1seg[:, s, :] = i1[r[:, None], (c[:, None] * SEG) + np.arange(SEG)]
            img2halo[:, s, :] = i2[
                r[:, None], (c[:, None] * SEG) + np.arange(SEG + HALO)
            ]
        packed = np.concatenate(
            [img1seg.reshape(128, IMG1_COLS), img2halo.reshape(128, HALO_COLS)],
            axis=1,
        )
        maps.append({"images": np.ascontiguousarray(packed)})
    return maps


def unshard_output(per_core):
    out = np.empty((N, D * C, H, W), dtype=np.float32)
    for k in range(NCORES):
        n = (k * ROWS) // H
        y0 = (k * ROWS) % H
        a = per_core[k].reshape(SLOTS, 4, 32, D, SEG)  # s, c, rm, d, x
        # rows r = 32*s + rm ; cols = c*SEG + x
        blk = a.transpose(3, 0, 2, 1, 4).reshape(D, ROWS, W)  # d, (s,rm), (c,x)
        out[n, :, y0 : y0 + ROWS, :] = blk
    return out


def kernel(image1, image2):
    nc = get_program()
    res = run_bass_kernel_spmd(nc, shard_inputs(image1, image2), list(range(NCORES)))
    return unshard_output([res.results[k]["out"] for k in range(NCORES)])
